# revision 5
# baseline (speedup 1.0000x reference)
"""Multi-head attention forward (B=2, N=2048, C=1024, H=16) on 8 TRN2 NeuronCores.

Tensor-parallel over heads: core c owns heads {2c, 2c+1}. Each core computes
QKV projection for its heads, full attention for its 4 (batch, head)
instances, and a partial output projection against its 128 rows of w_proj.
The host sums the 8 partial projections and adds the bias (row-parallel TP;
the all-reduce is the host-side unshard).

Per-core layouts (all matmul inputs bf16, PSUM accumulation f32):
  xT    [1024, 4096]  x^T, channel-major (replicated)
  wqk   [1024, 256]   [Wq_h0|Wq_h1|Wk_h0|Wk_h1] columns, Wq pre-scaled by D^-0.5
  wv    [1024, 128]   [Wv_h0|Wv_h1]
  wproj [128, 1024]   rows 128c:128c+128 of w_proj
  out   [4096, 1024]  f32 partial projection output

Attention per (b, h): S^T tiles [k=128, q=512] = KT_chunk.T @ QT (d-major, no
transposes), exp on ACT directly from PSUM, then O accumulation
exp(S^T).T @ [V|1] -- the appended ones-column accumulates the softmax
denominator in the same PSUM tile. Per-partition 1/l normalize, PE-transpose
of [q,d]->[d,q], then the partial projection.
"""

import numpy as np
import ml_dtypes

import concourse.bass as bass
import concourse.tile as tile
from concourse import bacc, mybir
from concourse.bass_utils import run_bass_kernel_spmd
from concourse.masks import make_identity

B, N, C = 2, 2048, 1024
H = 16
D = C // H          # 64
SCALE = D ** -0.5
NCORES = 8
T = B * N           # 4096 tokens
KT = C // 128       # 8 k-tiles over the C contraction
TOK_TILES = T // 128  # 32
NK = N // 128       # 16 key tiles per sequence
QB = 512            # q block width
NQB = N // QB       # 4
BF = mybir.dt.bfloat16
F32 = mybir.dt.float32

_NC_CACHE = {}


def build():
    nc = bacc.Bacc("TRN2", target_bir_lowering=False, debug=False,
                   num_devices=NCORES)
    xT = nc.dram_tensor("xT", [C, T], BF, kind="ExternalInput").ap()
    wqk = nc.dram_tensor("wqk", [C, 256], BF, kind="ExternalInput").ap()
    wv = nc.dram_tensor("wv", [C, 128], BF, kind="ExternalInput").ap()
    wproj = nc.dram_tensor("wproj", [128, C], BF, kind="ExternalInput").ap()
    out = nc.dram_tensor("out", [T, C], F32, kind="ExternalOutput").ap()

    with tile.TileContext(nc) as tc:
        with tc.tile_pool(name="const", bufs=1) as const, \
             tc.tile_pool(name="work", bufs=3) as work, \
             tc.tile_pool(name="obuf", bufs=2) as obuf_pool, \
             tc.tile_pool(name="ps", bufs=2, space="PSUM") as ps:

            xt_sb = const.tile([128, KT, T], BF, tag="xt")
            wqk_sb = const.tile([128, KT, 256], BF, tag="wqk")
            wv_sb = const.tile([128, KT, 128], BF, tag="wv")
            wproj_sb = const.tile([128, C], BF, tag="wproj")
            qk_sb = const.tile([128, 2, T], BF, tag="qk")   # [qchan|kchan, token]
            v_sb = const.tile([128, TOK_TILES, 130], BF, tag="v")  # [Vh0|1|Vh1|1]
            ot_sb = const.tile([128, T], BF, tag="ot")      # O^T [dchan, token]
            ident = const.tile([128, 128], BF, tag="ident")

            make_identity(nc, ident[:])

            for kt in range(KT):
                nc.sync.dma_start(out=xt_sb[:, kt, :],
                                  in_=xT[kt * 128:(kt + 1) * 128, :])
                nc.sync.dma_start(out=wqk_sb[:, kt, :],
                                  in_=wqk[kt * 128:(kt + 1) * 128, :])
                nc.sync.dma_start(out=wv_sb[:, kt, :],
                                  in_=wv[kt * 128:(kt + 1) * 128, :])
            nc.sync.dma_start(out=wproj_sb[:], in_=wproj[:, :])

            nc.vector.memset(v_sb[:, :, 64:65], 1.0)
            nc.vector.memset(v_sb[:, :, 129:130], 1.0)

            # ---- Stage 1a: [Q|K]^T = wqk.T @ x -> qk_sb [256, T] d-major ----
            for mt in range(2):
                for nt in range(T // 512):
                    pmm = ps.tile([128, 512], F32, tag="mm")
                    for kt in range(KT):
                        nc.tensor.matmul(
                            pmm[:],
                            wqk_sb[:, kt, mt * 128:(mt + 1) * 128],
                            xt_sb[:, kt, nt * 512:(nt + 1) * 512],
                            start=(kt == 0), stop=(kt == KT - 1))
                    nc.vector.tensor_copy(
                        qk_sb[:, mt, nt * 512:(nt + 1) * 512], pmm[:])

            # ---- Stage 1b: V = x @ wv -> v_sb token-major ----
            for t in range(TOK_TILES):
                pv = ps.tile([128, 128], F32, tag="mm")
                for kt in range(KT):
                    nc.tensor.matmul(
                        pv[:],
                        xt_sb[:, kt, t * 128:(t + 1) * 128],
                        wv_sb[:, kt, :],
                        start=(kt == 0), stop=(kt == KT - 1))
                nc.scalar.copy(v_sb[:, t, 0:64], pv[:, 0:64])
                nc.scalar.copy(v_sb[:, t, 65:129], pv[:, 64:128])

            # ---- Stage 2 + 3, per batch ----
            for b in range(B):
                o_sb = obuf_pool.tile([128, NK, 128], BF, tag="o")
                for h in range(2):
                    hp = slice(h * 64, (h + 1) * 64)
                    for qb in range(NQB):
                        q0 = b * N + qb * QB
                        po = ps.tile([128, 4 * 65], F32, tag="o")
                        es_prev = None
                        kp_prev = None
                        for kp in range(NK // 2):
                            pst = ps.tile([128, 1024], F32, tag="s")
                            for j in range(2):
                                k0 = b * N + (kp * 2 + j) * 128
                                nc.tensor.matmul(
                                    pst[:, j * 512:(j + 1) * 512],
                                    qk_sb[hp, 1, k0:k0 + 128],
                                    qk_sb[hp, 0, q0:q0 + QB],
                                    start=True, stop=True)
                            if es_prev is not None:
                                self_o_mms(nc, po, es_prev, v_sb, b, kp_prev, h,
                                           first=(kp_prev == 0), last=False)
                            es = work.tile([128, 1024], BF, tag="es")
                            nc.scalar.activation(
                                es[:], pst[:], mybir.ActivationFunctionType.Exp)
                            es_prev, kp_prev = es, kp
                        self_o_mms(nc, po, es_prev, v_sb, b, kp_prev, h,
                                   first=(kp_prev == 0), last=True)
                        # normalize: O[q, d] /= l[q]
                        for qs in range(4):
                            qsub = qb * 4 + qs
                            linv = work.tile([128, 1], F32, tag="linv")
                            nc.vector.reciprocal(
                                linv[:], po[:, qs * 65 + 64:qs * 65 + 65])
                            nc.vector.tensor_scalar_mul(
                                o_sb[:, qsub, h * 64:(h + 1) * 64],
                                po[:, qs * 65:qs * 65 + 64], linv[:])
                # transpose O [q, d] -> O^T [d, q] per 128-token tile
                for qsub in range(NK):
                    pt = ps.tile([128, 128], BF, tag="mm")
                    nc.tensor.transpose(pt[:], o_sb[:, qsub, :], ident[:])
                    nc.vector.tensor_copy(
                        ot_sb[:, b * N + qsub * 128:b * N + (qsub + 1) * 128],
                        pt[:])
                # ---- Stage 3: partial projection for this batch ----
                for g in range(b * NK, (b + 1) * NK):
                    ob = work.tile([128, C], F32, tag="outstage")
                    for ntile in range(2):
                        pmm = ps.tile([128, 512], F32, tag="mm")
                        nc.tensor.matmul(
                            pmm[:],
                            ot_sb[:, g * 128:(g + 1) * 128],
                            wproj_sb[:, ntile * 512:(ntile + 1) * 512],
                            start=True, stop=True)
                        if ntile == 0:
                            nc.vector.tensor_copy(ob[:, 0:512], pmm[:])
                        else:
                            nc.scalar.copy(ob[:, 512:1024], pmm[:])
                    nc.sync.dma_start(
                        out=out[g * 128:(g + 1) * 128, :], in_=ob[:])
    nc.compile()
    return nc


def self_o_mms(nc, po, es, v_sb, b, kp, h, first, last):
    """O += exp(S^T).T @ [V|1] for the two k-tiles held in es.

    The po bank holds 4 independent 65-col accumulation regions; PSUM zero
    regions are 2KB-granular, so the whole bank is one start/stop group:
    start marks the bank pending-zero, each region's first write overwrites,
    later writes accumulate.
    """
    for j in range(2):
        vt = b * NK + kp * 2 + j
        for qs in range(4):
            nc.tensor.matmul(
                po[:, qs * 65:(qs + 1) * 65],
                es[:, j * 512 + qs * 128:j * 512 + (qs + 1) * 128],
                v_sb[:, vt, h * 65:(h + 1) * 65],
                start=(first and j == 0 and qs == 0),
                stop=(last and j == 1 and qs == 3))


def make_in_maps(x, w_qkv, w_proj):
    bf = ml_dtypes.bfloat16
    x2 = x.reshape(T, C)
    xT_np = np.ascontiguousarray(x2.T).astype(bf)
    in_maps = []
    for c in range(NCORES):
        s = c * 128
        wq = w_qkv[:, s:s + 128] * SCALE
        wk = w_qkv[:, C + s:C + s + 128]
        wqk_np = np.ascontiguousarray(
            np.concatenate([wq, wk], axis=1)).astype(bf)
        wv_np = np.ascontiguousarray(
            w_qkv[:, 2 * C + s:2 * C + s + 128]).astype(bf)
        wproj_np = np.ascontiguousarray(w_proj[s:s + 128, :]).astype(bf)
        in_maps.append({"xT": xT_np, "wqk": wqk_np, "wv": wv_np,
                        "wproj": wproj_np})
    return in_maps


def kernel(x, w_qkv, w_proj, b_proj):
    x = np.asarray(x, dtype=np.float32)
    w_qkv = np.asarray(w_qkv, dtype=np.float32)
    w_proj = np.asarray(w_proj, dtype=np.float32)
    b_proj = np.asarray(b_proj, dtype=np.float32)

    if "nc" not in _NC_CACHE:
        _NC_CACHE["nc"] = build()
    nc = _NC_CACHE["nc"]

    in_maps = make_in_maps(x, w_qkv, w_proj)
    res = run_bass_kernel_spmd(nc, in_maps, list(range(NCORES)))
    acc = np.zeros((T, C), dtype=np.float32)
    for r in res.results:
        acc += r["out"]
    acc += b_proj[None, :]
    return acc.reshape(B, N, C)
